# revision 36
# baseline (speedup 1.0000x reference)
"""Binary-tree gated-expert MoE (root -> 2 mid -> 4 leaf experts) on 8 trn2 cores.

Strategy: expert-parallel dispatch by leaf index. Tokens are grouped on the
host by their 2-bit routing path (leaf = 2*bit0 + bit1); each of the 8
NeuronCores processes one contiguous chunk of one leaf's tokens (cores are
apportioned to leaves proportionally to token counts, 2 cores/leaf in the
balanced case). A core then runs 3 chained dense [C,2048]x[2048,2048] layers
(root W0, mid W1[bit0], leaf W2[leaf]) with relu+bias, entirely on-chip.

Device kernel keeps activations transposed ([D, tokens] feature-major) so each
layer's matmul output (PSUM [fout, tok]) is directly the next layer's rhs.
Matmuls run in fp16 (same TensorE rate as bf16, 8x finer mantissa) with fp32
PSUM accumulation; weights stream from HBM as pre-tiled [16, 128, 2048]
stripes used as the stationary operand.

Head: x is shipped as the exact SBUF image ([128, KT*C] fp16) so input DMAs
are long contiguous runs (2082B+ per descriptor, ~360GB/s aggregate) split
into k-blocks interleaved across both DMA rings in need-order (each dma_start
trigger costs ~590ns of engine time, so few/bigger is better). ~36
dependency-free warm-up matmuls run during the framework preamble so the PE's
HAM clock-gate is mostly warm when the real stream starts. Layer 0 runs a
streaming phase A: the first 8 (m,n) tiles accumulate k-outer across all 8
PSUM banks, so each incoming x chunk is consumed by 8 matmuls (~1.2us) --
matching DMA delivery rate -- and the PE never starves or HAM-rethrottles
while x streams in. The remaining tiles and layers 1-2 run k-inner per (m,n)
tile so each tile's epilogue and (final layer) per-m fused out-DMA fire as
soon as accumulation completes. All weight-stripe and fused-y triggers ride
the sync engine so ACT epilogues are never queue-blocked behind them; the
last m's output is per-tile with a small 64-column final piece so the
end-of-stream chain (epilogue + trigger + DMA) is minimal. Output is written
fp16 (absmax err 5.8e-4 vs the 2e-2 gate).
"""

import numpy as np
from contextlib import ExitStack

import concourse.bass as bass
from concourse import bacc, mybir, tile
from concourse.bass_utils import run_bass_kernel_spmd


def _ensure_ntff_hook():
    """bass_utils' trace path does `from antenv.axon_hooks import ...` at call
    time; some images ship an antenv without that submodule, which would crash
    the run when BASS_TRACE is set. If (and only if) the import fails, register
    an equivalent module backed by the libaxon ctypes NTFF interface (mirrors
    trn_agent_boot.trn_boot). Inert when the real module exists."""
    try:
        from antenv.axon_hooks import get_axon_ntff_profile_hook  # noqa: F401
        return
    except ImportError:
        pass
    import sys, types, ctypes, contextlib

    mod = types.ModuleType("antenv.axon_hooks")
    holder = [None]
    mod.set_axon_ntff_profile_hook = lambda h: holder.__setitem__(0, h)
    mod.get_axon_ntff_profile_hook = lambda: holder[0]
    sys.modules["antenv.axon_hooks"] = mod
    try:
        import antenv

        antenv.axon_hooks = mod
    except ImportError:
        pass
    try:
        lib = ctypes.CDLL("/opt/axon/libaxon_pjrt.so")
    except OSError:
        return
    if not hasattr(lib, "axon_start_nrt_profile"):
        return
    lib.axon_start_nrt_profile.argtypes = [
        ctypes.POINTER(ctypes.c_int64),
        ctypes.c_size_t,
    ]
    lib.axon_start_nrt_profile.restype = ctypes.c_int64
    lib.axon_stop_nrt_profile.argtypes = [ctypes.c_char_p]
    lib.axon_stop_nrt_profile.restype = ctypes.c_int64

    @contextlib.contextmanager
    def _hook(output_dir, device_ids):
        import jax

        jax.devices()
        if device_ids:
            ids = (ctypes.c_int64 * len(device_ids))(*device_ids)
            rc = lib.axon_start_nrt_profile(ids, len(device_ids))
        else:
            rc = lib.axon_start_nrt_profile(None, 0)
        if rc != 0:
            raise RuntimeError(f"axon_start_nrt_profile rc={rc}")
        try:
            yield
        finally:
            n = lib.axon_stop_nrt_profile(str(output_dir).encode())
            print(f"profile: {n} ntff file(s) written to {output_dir}")

    mod.set_axon_ntff_profile_hook(_hook)


_ensure_ntff_hook()

D = 2048
PT = 128           # partition tile
KT = D // PT       # 16 contraction tiles per layer
MT = D // PT       # 16 output-feature tiles per layer
N_CORES = 8
MH = 3             # head stripes (m0..m2) shipped packed for phase A

F32 = mybir.dt.float32
F16 = mybir.dt.float16
NP_F16 = np.float16

# cache of compiled bass programs keyed by padded capacity C
_compiled = {}
# stash of the last run's results so a harness can inspect exec_time_ns
last_results = None


def _prep_weight(W):
    """[D, D] -> [MT, 128, D] fp16: stripe m holds W[:, m*128:(m+1)*128]
    rearranged so partition p = contraction row within k-chunk, and the free
    dim is (k, fout-col) — i.e. out[m, p, k*128 + c] = W[k*128 + p, m*128 + c].
    Each [128, 2048] stripe then DMAs contiguously into SBUF and its k-th
    [128, 128] column block is exactly the lhsT (stationary) matmul operand."""
    W4 = W.reshape(KT, PT, MT, PT)
    return np.ascontiguousarray(
        W4.transpose(2, 1, 0, 3).reshape(MT, PT, D).astype(NP_F16)
    )


def _prep_bias(b0, b1e, b2l):
    """three [D] biases -> [128, 3*MT] f32 where column li*MT + m holds
    bias[li][m*128 : (m+1)*128] along partitions."""
    cols = []
    for b in (b0, b1e, b2l):
        cols.append(b.reshape(MT, PT).T)  # [128, MT]
    return np.ascontiguousarray(np.concatenate(cols, axis=1).astype(np.float32))


def _tiling(maxg):
    """Pick (TN, NT, C): NT token tiles of TN columns, C = NT*TN >= maxg,
    TN <= 512 (one PSUM bank of fp32), minimizing padded capacity C."""
    maxg = max(maxg, 128)
    NT = -(-maxg // 512)
    TN = -(-maxg // NT)
    return TN, NT, TN * NT


def _build(C, TN, NT):
    """Build + compile the 3-layer SPMD program for per-core capacity C."""
    nc = bacc.Bacc(
        "TRN2",
        target_bir_lowering=False,
        debug=False,
        enable_asserts=False,
        num_devices=N_CORES,
    )
    # x shipped as the SBUF image: xP[p, k*C + c] = x^T[k*128 + p, c].
    # Any [ka:kb) chunk range is then one DMA with (kb-ka)*C*2B contiguous
    # per-partition runs.
    xP = nc.dram_tensor("xP", [PT, KT * C], F16, kind="ExternalInput").ap()
    # first MH stripes of W0, k-sliced so the head can stream them in lockstep
    # with x: w0H[p, (k*MH + m)*PT + c] = prep_w0[m][p][k*PT + c]
    w0H = nc.dram_tensor("w0H", [PT, KT * MH * PT], F16, kind="ExternalInput").ap()
    w0 = nc.dram_tensor("w0", [MT, PT, D], F16, kind="ExternalInput").ap()
    w1 = nc.dram_tensor("w1", [MT, PT, D], F16, kind="ExternalInput").ap()
    w2 = nc.dram_tensor("w2", [MT, PT, D], F16, kind="ExternalInput").ap()
    bias = nc.dram_tensor("bias", [PT, 3 * MT], F32, kind="ExternalInput").ap()
    # y staged as [p, m, token]: per-m out-DMA is one [128, NT*TN*2B] run
    yS = nc.dram_tensor("yS", [PT, MT, NT * TN], F16, kind="ExternalOutput").ap()

    with tile.TileContext(nc) as tc, ExitStack() as ctx:
        wpool = ctx.enter_context(tc.tile_pool(name="w", bufs=4))
        hpool = ctx.enter_context(tc.tile_pool(name="h", bufs=1))
        pspool = ctx.enter_context(tc.tile_pool(name="ps", bufs=8, space="PSUM"))
        opool = ctx.enter_context(tc.tile_pool(name="o", bufs=3))
        cpool = ctx.enter_context(tc.tile_pool(name="c", bufs=1))

        hA = hpool.tile([PT, KT, C], F16, tag="hA")
        hB = hpool.tile([PT, KT, C], F16, tag="hB")
        wH = cpool.tile([PT, KT, MH, PT], F16, tag="wH")
        bias_sb = cpool.tile([PT, 3 * MT], F32)
        warm_w = cpool.tile([PT, PT], F16, tag="warm")
        warm_ps = pspool.tile([PT, 64], F32, tag="ps", name="warm_ps")

        # ---- PE pre-warm ----
        # ~36 dependency-free matmuls run during the framework preamble and
        # the first input DMAs, warming the PE's HAM clock-gate toward 8/8
        # before the real matmul stream starts. Sized to ~2us: a longer warm
        # delays the real stream past what the x input DMA can hide (the
        # cold-rate ramp and the x-delivery wait overlap, so covering the
        # full 3.4us HAM window is a net loss).
        nc.vector.memset(warm_w[:], 0.0)
        for _ in range(36):
            nc.tensor.matmul(
                warm_ps[:], warm_w[:], warm_w[:, 0:64],
                start=True, stop=True,
            )

        # ---- head DMAs ----
        # x chunks and the phase-A weight k-slices are interleaved across
        # both rings in need-order (blocks of 2 k's, alternating rings; w0H
        # blocks ride the opposite ring from the same k's x), so both rings
        # deliver the streaming phase-A operands in lockstep with consumption.
        w0Hk = w0H.rearrange("p (k m c) -> p k m c", k=KT, m=MH)
        nc.scalar.dma_start(wH[:, 0, 0], w0Hk[:, 0, 0])
        nc.sync.dma_start(hA[:, 0, 0:TN], xP[:, 0:TN])
        nc.scalar.dma_start(wH[:, 0, 1:MH], w0Hk[:, 0, 1:MH])
        nc.sync.dma_start(hA[:, 0, TN:C], xP[:, TN:C])
        nc.scalar.dma_start(wH[:, 1:3], w0Hk[:, 1:3])
        # k1-k6 gated per n-tile: the PE resumes within one ~90KB sub-piece
        # of delivery instead of stalling for a whole chunk (a >1us stall
        # also breaks the HAM busy window and prolongs the cold ramp)
        for k in range(1, 7):
            xeng = nc.sync if k % 2 == 1 else nc.scalar
            for n in range(NT):
                c0, c1 = n * TN, min((n + 1) * TN, C)
                xeng.dma_start(hA[:, k, c0:c1], xP[:, k * C + c0 : k * C + c1])
            if k >= 3:
                weng = nc.scalar if k % 2 == 1 else nc.sync
                weng.dma_start(wH[:, k : k + 1], w0Hk[:, k : k + 1])
        kblocks = [(k, min(k + 2, KT)) for k in range(7, KT, 2)]
        for bi, (ka, kb) in enumerate(kblocks):
            xeng, weng = (nc.sync, nc.scalar) if bi % 2 == 0 else (nc.scalar, nc.sync)
            weng.dma_start(wH[:, ka:kb], w0Hk[:, ka:kb])
            xeng.dma_start(
                hA[:, ka:kb, :],
                xP[:, ka * C : kb * C].rearrange("p (k c) -> p k c", k=kb - ka),
            )
        nc.sync.dma_start(bias_sb[:], bias[:])

        def relu_bias(out_ap, ps_ap, b_ap, on_dve):
            if on_dve:
                nc.vector.tensor_scalar(
                    out_ap, ps_ap, b_ap, 0.0,
                    mybir.AluOpType.add, mybir.AluOpType.max,
                )
            else:
                nc.scalar.activation(
                    out_ap, ps_ap,
                    mybir.ActivationFunctionType.Relu, bias=b_ap,
                )

        # ---- layer 0, phase A: first 8 (m,n) tiles, k-outer across all 8
        # PSUM banks; each x chunk is consumed by 8 matmuls as it lands ----
        pa_tiles = []          # (m, n) in n-inner order, capped at 8 banks
        for m in range(min(MT, MH)):   # phase A only has wH stripes m0..MH-1
            for n in range(NT):
                if len(pa_tiles) < 8:
                    pa_tiles.append((m, n))
        pa_ms = sorted({m for m, _ in pa_tiles})
        psA = {
            (m, n): pspool.tile([PT, TN], F32, tag="ps", name=f"psA_{m}_{n}")
            for m, n in pa_tiles
        }
        # n-outer within k so the first matmuls gate on the small k0/n0 piece
        pa_order = sorted(pa_tiles, key=lambda t: (t[1], t[0]))
        for k in range(KT):
            for m, n in pa_order:
                nc.tensor.matmul(
                    psA[(m, n)][:],
                    wH[:, k, m, :],
                    hA[:, k, bass.ts(n, TN)],
                    start=(k == 0),
                    stop=(k == KT - 1),
                    skip_group_check=True,
                )
        ei = 0
        for m, n in pa_tiles:
            relu_bias(hB[:, m, bass.ts(n, TN)], psA[(m, n)][:],
                      bias_sb[:, m : m + 1], ei % 2 == 1)
            ei += 1

        # ---- layer 0, phase B: remaining tiles k-inner (x resident) ----
        # finish partially-covered phase-A m's, then the rest
        done = set(pa_tiles)
        for m in pa_ms:
            for n in range(NT):
                if (m, n) in done:
                    continue
                ps = pspool.tile([PT, TN], F32, tag="ps", name=f"ps0_{m}_{n}")
                for k in range(KT):
                    nc.tensor.matmul(
                        ps[:], wH[:, k, m, :],
                        hA[:, k, bass.ts(n, TN)],
                        start=(k == 0), stop=(k == KT - 1),
                    )
                relu_bias(hB[:, m, bass.ts(n, TN)], ps[:],
                          bias_sb[:, m : m + 1], ei % 2 == 1)
                ei += 1
        for m in range(len(pa_ms), MT):
            wt = wpool.tile([PT, D], F16, tag="wt", name=f"wt0_{m}")
            nc.sync.dma_start(wt[:], w0[m])
            for n in range(NT):
                ps = pspool.tile([PT, TN], F32, tag="ps", name=f"ps0_{m}_{n}")
                for k in range(KT):
                    nc.tensor.matmul(
                        ps[:], wt[:, k * PT : (k + 1) * PT],
                        hA[:, k, bass.ts(n, TN)],
                        start=(k == 0), stop=(k == KT - 1),
                    )
                relu_bias(hB[:, m, bass.ts(n, TN)], ps[:],
                          bias_sb[:, m : m + 1], (n + m) % 2 == 1)

        # ---- layers 1 and 2 ----
        for w_dram, li, h_in, h_out in ((w1, 1, hB, hA), (w2, 2, hA, None)):
            for m in range(MT):
                wt = wpool.tile([PT, D], F16, tag="wt", name=f"wt{li}_{m}")
                nc.sync.dma_start(wt[:], w_dram[m])
                b_ap = bias_sb[:, li * MT + m : li * MT + m + 1]
                last_m = li == 2 and m == MT - 1
                fine_m = li == 2 and m >= MT - 3
                if h_out is None and not fine_m:
                    ot = opool.tile([PT, NT * TN], F16, tag="ot", name=f"ot{m}")
                for n in range(NT):
                    if last_m and n == NT - 1:
                        # kernel tail: final tile split unevenly; the last
                        # piece is small and rides a by-then-empty ring so
                        # the end-of-stream chain (epilogue + trigger + DMA)
                        # is as short as possible
                        cut = max(TN - 32, TN // 2)
                        for hi, (c0, c1) in enumerate(((0, cut), (cut, TN))):
                            psH = pspool.tile(
                                [PT, c1 - c0], F32, tag="ps", name=f"ps_last{hi}"
                            )
                            for k in range(KT):
                                nc.tensor.matmul(
                                    psH[:], wt[:, k * PT : (k + 1) * PT],
                                    h_in[:, k, n * TN + c0 : n * TN + c1],
                                    start=(k == 0), stop=(k == KT - 1),
                                )
                            otH = opool.tile([PT, c1 - c0], F16, tag="ot",
                                             name=f"ot_last{hi}")
                            relu_bias(otH[:], psH[:], b_ap, hi == 1)
                            dma_eng = nc.scalar if hi == 0 else nc.sync
                            dma_eng.dma_start(
                                yS[:, m, n * TN + c0 : n * TN + c1], otH[:]
                            )
                        continue
                    ps = pspool.tile([PT, TN], F32, tag="ps", name=f"ps{li}_{m}_{n}")
                    for k in range(KT):
                        nc.tensor.matmul(
                            ps[:], wt[:, k * PT : (k + 1) * PT],
                            h_in[:, k, bass.ts(n, TN)],
                            start=(k == 0), stop=(k == KT - 1),
                        )
                    on_dve = (n + m) % 2 == 1
                    if h_out is not None:
                        relu_bias(h_out[:, m, bass.ts(n, TN)], ps[:], b_ap, on_dve)
                    elif fine_m:
                        # per-tile out-DMA near the end so the final y data
                        # drains early instead of piling into the tail
                        otN = opool.tile([PT, TN], F16, tag="ot", name=f"otN{m}_{n}")
                        relu_bias(otN[:], ps[:], b_ap, on_dve)
                        dma_eng = nc.scalar if (last_m and n == 1) else nc.sync
                        dma_eng.dma_start(yS[:, m, bass.ts(n, TN)], otN[:])
                    else:
                        relu_bias(ot[:, bass.ts(n, TN)], ps[:], b_ap, on_dve)
                if h_out is None and not fine_m:
                    # per-m fused out-DMA: one [128, NT*TN*2B] contiguous run
                    nc.sync.dma_start(yS[:, m, :], ot[:])
    nc.compile()
    return nc


def _apportion_cores(counts):
    """Assign 8 cores to 4 leaves ~proportionally to token counts.
    Returns list of core counts per leaf (sums to N_CORES; 0 only for empty
    leaves). Greedy: repeatedly hand a core to the leaf with max load/core."""
    alive = [l for l in range(4) if counts[l] > 0]
    n = {l: 1 for l in alive}
    for _ in range(N_CORES - len(alive)):
        l = max(alive, key=lambda l: counts[l] / n[l])
        n[l] += 1
    return [n.get(l, 0) for l in range(4)]


def kernel(x, W0, b0, W1, b1, W2, b2, path_mask):
    global last_results
    x = np.asarray(x, dtype=np.float32)
    path_mask = np.asarray(path_mask)
    W0, b0, W1, b1, W2, b2 = (
        np.asarray(a, dtype=np.float32) for a in (W0, b0, W1, b1, W2, b2)
    )
    B = x.shape[0]

    bit0 = path_mask[:, 0].astype(np.int64)
    bit1 = path_mask[:, 1].astype(np.int64)
    leaf = 2 * bit0 + bit1
    order = np.argsort(leaf, kind="stable")
    counts = np.bincount(leaf, minlength=4)

    per_leaf = _apportion_cores(counts)
    # contiguous chunks of the leaf-sorted order per core
    groups = []      # list of (leaf, index-array) per core
    start = 0
    for l in range(4):
        cnt = int(counts[l])
        tok = order[start : start + cnt]
        start += cnt
        nl = per_leaf[l]
        if nl == 0:
            continue
        bounds = [round(i * cnt / nl) for i in range(nl + 1)]
        for i in range(nl):
            groups.append((l, tok[bounds[i] : bounds[i + 1]]))
    while len(groups) < N_CORES:  # only if some leaf was empty and slots remain
        groups.append((0, np.zeros(0, dtype=np.int64)))

    maxg = max(len(g[1]) for g in groups)
    TN, NT, C = _tiling(maxg)

    if C not in _compiled:
        _compiled[C] = _build(C, TN, NT)
    nc = _compiled[C]

    w_prepped = {}  # cache per (matrix id)
    def wp(tag, W):
        if tag not in w_prepped:
            w_prepped[tag] = _prep_weight(W)
        return w_prepped[tag]

    w0p = wp("w0", W0)
    # w0H[p, (k*MH + m)*PT + c] = w0p[m, p, k*PT + c]
    w0H = np.ascontiguousarray(
        w0p[:MH].reshape(MH, PT, KT, PT).transpose(1, 2, 0, 3).reshape(PT, KT * MH * PT)
    )
    xb = x.astype(NP_F16)
    in_maps = []
    for l, tok in groups:
        xTg = np.zeros((D, C), dtype=NP_F16)
        if len(tok):
            xTg[:, : len(tok)] = xb[tok].T
        xPg = np.ascontiguousarray(
            xTg.reshape(KT, PT, C).transpose(1, 0, 2).reshape(PT, KT * C)
        )
        in_maps.append(
            {
                "xP": xPg,
                "w0H": w0H,
                "w0": w0p,
                "w1": wp(("w1", l // 2), W1[l // 2]),
                "w2": wp(("w2", l), W2[l]),
                "bias": _prep_bias(b0, b1[l // 2], b2[l]),
            }
        )

    last_results = run_bass_kernel_spmd(nc, in_maps, core_ids=list(range(N_CORES)))

    y = np.empty((B, D), dtype=np.float32)
    for (l, tok), res in zip(groups, last_results.results):
        if len(tok):
            # [PT, MT, NT*TN] -> [D, C]
            yT = res["yS"].transpose(1, 0, 2).reshape(D, C)
            y[tok] = yT[:, : len(tok)].T.astype(np.float32)
    return y


# revision 37
# speedup vs baseline: 1.0055x; 1.0055x over previous
"""Binary-tree gated-expert MoE (root -> 2 mid -> 4 leaf experts) on 8 trn2 cores.

Strategy: expert-parallel dispatch by leaf index. Tokens are grouped on the
host by their 2-bit routing path (leaf = 2*bit0 + bit1); each of the 8
NeuronCores processes one contiguous chunk of one leaf's tokens (cores are
apportioned to leaves proportionally to token counts, 2 cores/leaf in the
balanced case). A core then runs 3 chained dense [C,2048]x[2048,2048] layers
(root W0, mid W1[bit0], leaf W2[leaf]) with relu+bias, entirely on-chip.

Device kernel keeps activations transposed ([D, tokens] feature-major) so each
layer's matmul output (PSUM [fout, tok]) is directly the next layer's rhs.
Matmuls run in fp16 (same TensorE rate as bf16, 8x finer mantissa) with fp32
PSUM accumulation; weights stream from HBM as pre-tiled [16, 128, 2048]
stripes used as the stationary operand.

Head: x is shipped as the exact SBUF image ([128, KT*C] fp16) so input DMAs
are long contiguous runs (2082B+ per descriptor, ~360GB/s aggregate) split
into k-blocks interleaved across both DMA rings in need-order (each dma_start
trigger costs ~590ns of engine time, so few/bigger is better). ~36
dependency-free warm-up matmuls run during the framework preamble so the PE's
HAM clock-gate is mostly warm when the real stream starts. Layer 0 runs a
streaming phase A: the first 8 (m,n) tiles accumulate k-outer across all 8
PSUM banks, so each incoming x chunk is consumed by 8 matmuls (~1.2us) --
matching DMA delivery rate -- and the PE never starves or HAM-rethrottles
while x streams in. The remaining tiles and layers 1-2 run k-inner per (m,n)
tile so each tile's epilogue and (final layer) per-m fused out-DMA fire as
soon as accumulation completes. All weight-stripe and fused-y triggers ride
the sync engine so ACT epilogues are never queue-blocked behind them; the
last m's output is per-tile with a small 64-column final piece so the
end-of-stream chain (epilogue + trigger + DMA) is minimal. Output is written
fp16 (absmax err 5.8e-4 vs the 2e-2 gate).
"""

import numpy as np
from contextlib import ExitStack

import concourse.bass as bass
from concourse import bacc, mybir, tile
from concourse.bass_utils import run_bass_kernel_spmd


def _ensure_ntff_hook():
    """bass_utils' trace path does `from antenv.axon_hooks import ...` at call
    time; some images ship an antenv without that submodule, which would crash
    the run when BASS_TRACE is set. If (and only if) the import fails, register
    an equivalent module backed by the libaxon ctypes NTFF interface (mirrors
    trn_agent_boot.trn_boot). Inert when the real module exists."""
    try:
        from antenv.axon_hooks import get_axon_ntff_profile_hook  # noqa: F401
        return
    except ImportError:
        pass
    import sys, types, ctypes, contextlib

    mod = types.ModuleType("antenv.axon_hooks")
    holder = [None]
    mod.set_axon_ntff_profile_hook = lambda h: holder.__setitem__(0, h)
    mod.get_axon_ntff_profile_hook = lambda: holder[0]
    sys.modules["antenv.axon_hooks"] = mod
    try:
        import antenv

        antenv.axon_hooks = mod
    except ImportError:
        pass
    try:
        lib = ctypes.CDLL("/opt/axon/libaxon_pjrt.so")
    except OSError:
        return
    if not hasattr(lib, "axon_start_nrt_profile"):
        return
    lib.axon_start_nrt_profile.argtypes = [
        ctypes.POINTER(ctypes.c_int64),
        ctypes.c_size_t,
    ]
    lib.axon_start_nrt_profile.restype = ctypes.c_int64
    lib.axon_stop_nrt_profile.argtypes = [ctypes.c_char_p]
    lib.axon_stop_nrt_profile.restype = ctypes.c_int64

    @contextlib.contextmanager
    def _hook(output_dir, device_ids):
        import jax

        jax.devices()
        if device_ids:
            ids = (ctypes.c_int64 * len(device_ids))(*device_ids)
            rc = lib.axon_start_nrt_profile(ids, len(device_ids))
        else:
            rc = lib.axon_start_nrt_profile(None, 0)
        if rc != 0:
            raise RuntimeError(f"axon_start_nrt_profile rc={rc}")
        try:
            yield
        finally:
            n = lib.axon_stop_nrt_profile(str(output_dir).encode())
            print(f"profile: {n} ntff file(s) written to {output_dir}")

    mod.set_axon_ntff_profile_hook(_hook)


_ensure_ntff_hook()

D = 2048
PT = 128           # partition tile
KT = D // PT       # 16 contraction tiles per layer
MT = D // PT       # 16 output-feature tiles per layer
N_CORES = 8
MH = 3             # head stripes (m0..m2) shipped packed for phase A

F32 = mybir.dt.float32
F16 = mybir.dt.float16
NP_F16 = np.float16

# cache of compiled bass programs keyed by padded capacity C
_compiled = {}
# stash of the last run's results so a harness can inspect exec_time_ns
last_results = None


def _prep_weight(W):
    """[D, D] -> [MT, 128, D] fp16: stripe m holds W[:, m*128:(m+1)*128]
    rearranged so partition p = contraction row within k-chunk, and the free
    dim is (k, fout-col) — i.e. out[m, p, k*128 + c] = W[k*128 + p, m*128 + c].
    Each [128, 2048] stripe then DMAs contiguously into SBUF and its k-th
    [128, 128] column block is exactly the lhsT (stationary) matmul operand."""
    W4 = W.reshape(KT, PT, MT, PT)
    return np.ascontiguousarray(
        W4.transpose(2, 1, 0, 3).reshape(MT, PT, D).astype(NP_F16)
    )


def _prep_bias(b0, b1e, b2l):
    """three [D] biases -> [128, 3*MT] f32 where column li*MT + m holds
    bias[li][m*128 : (m+1)*128] along partitions."""
    cols = []
    for b in (b0, b1e, b2l):
        cols.append(b.reshape(MT, PT).T)  # [128, MT]
    return np.ascontiguousarray(np.concatenate(cols, axis=1).astype(np.float32))


def _tiling(maxg):
    """Pick (TN, NT, C): NT token tiles of TN columns, C = NT*TN >= maxg,
    TN <= 512 (one PSUM bank of fp32), minimizing padded capacity C."""
    maxg = max(maxg, 128)
    NT = -(-maxg // 512)
    TN = -(-maxg // NT)
    return TN, NT, TN * NT


def _build(C, TN, NT):
    """Build + compile the 3-layer SPMD program for per-core capacity C."""
    nc = bacc.Bacc(
        "TRN2",
        target_bir_lowering=False,
        debug=False,
        enable_asserts=False,
        num_devices=N_CORES,
    )
    # x shipped as the SBUF image: xP[p, k*C + c] = x^T[k*128 + p, c].
    # Any [ka:kb) chunk range is then one DMA with (kb-ka)*C*2B contiguous
    # per-partition runs.
    xP = nc.dram_tensor("xP", [PT, KT * C], F16, kind="ExternalInput").ap()
    # first MH stripes of W0, k-sliced so the head can stream them in lockstep
    # with x: w0H[p, (k*MH + m)*PT + c] = prep_w0[m][p][k*PT + c]
    w0H = nc.dram_tensor("w0H", [PT, KT * MH * PT], F16, kind="ExternalInput").ap()
    w0 = nc.dram_tensor("w0", [MT, PT, D], F16, kind="ExternalInput").ap()
    w1 = nc.dram_tensor("w1", [MT, PT, D], F16, kind="ExternalInput").ap()
    w2 = nc.dram_tensor("w2", [MT, PT, D], F16, kind="ExternalInput").ap()
    bias = nc.dram_tensor("bias", [PT, 3 * MT], F32, kind="ExternalInput").ap()
    # y staged as [p, m, token]: per-m out-DMA is one [128, NT*TN*2B] run
    yS = nc.dram_tensor("yS", [PT, MT, NT * TN], F16, kind="ExternalOutput").ap()

    with tile.TileContext(nc) as tc, ExitStack() as ctx:
        wpool = ctx.enter_context(tc.tile_pool(name="w", bufs=4))
        hpool = ctx.enter_context(tc.tile_pool(name="h", bufs=1))
        pspool = ctx.enter_context(tc.tile_pool(name="ps", bufs=8, space="PSUM"))
        opool = ctx.enter_context(tc.tile_pool(name="o", bufs=3))
        cpool = ctx.enter_context(tc.tile_pool(name="c", bufs=1))

        hA = hpool.tile([PT, KT, C], F16, tag="hA")
        hB = hpool.tile([PT, KT, C], F16, tag="hB")
        wH = cpool.tile([PT, KT, MH, PT], F16, tag="wH")
        bias_sb = cpool.tile([PT, 3 * MT], F32)
        warm_w = cpool.tile([PT, PT], F16, tag="warm")
        warm_ps = pspool.tile([PT, 64], F32, tag="ps", name="warm_ps")

        # ---- PE pre-warm ----
        # ~36 dependency-free matmuls run during the framework preamble and
        # the first input DMAs, warming the PE's HAM clock-gate toward 8/8
        # before the real matmul stream starts. Sized to ~2us: a longer warm
        # delays the real stream past what the x input DMA can hide (the
        # cold-rate ramp and the x-delivery wait overlap, so covering the
        # full 3.4us HAM window is a net loss).
        nc.vector.memset(warm_w[:], 0.0)
        for _ in range(36):
            nc.tensor.matmul(
                warm_ps[:], warm_w[:], warm_w[:, 0:64],
                start=True, stop=True,
            )

        # ---- head DMAs ----
        # x chunks and the phase-A weight k-slices are interleaved across
        # both rings in need-order (blocks of 2 k's, alternating rings; w0H
        # blocks ride the opposite ring from the same k's x), so both rings
        # deliver the streaming phase-A operands in lockstep with consumption.
        w0Hk = w0H.rearrange("p (k m c) -> p k m c", k=KT, m=MH)
        nc.scalar.dma_start(wH[:, 0:1], w0Hk[:, 0:1])
        nc.sync.dma_start(hA[:, 0, 0:TN], xP[:, 0:TN])
        nc.sync.dma_start(hA[:, 0, TN:C], xP[:, TN:C])
        nc.scalar.dma_start(wH[:, 1:3], w0Hk[:, 1:3])
        # k1/k2 gated per n-tile: the PE resumes within one ~90KB sub-piece
        # of delivery instead of stalling for a whole chunk (a >1us stall
        # also breaks the HAM busy window and prolongs the cold ramp).
        # Finer splitting of k3+ was measured WORSE (small-descriptor DMAs
        # slow aggregate delivery and starve the later chunks instead).
        for n in range(NT):
            c0, c1 = n * TN, min((n + 1) * TN, C)
            nc.sync.dma_start(hA[:, 1, c0:c1], xP[:, C + c0 : C + c1])
        for n in range(NT):
            c0, c1 = n * TN, min((n + 1) * TN, C)
            nc.scalar.dma_start(hA[:, 2, c0:c1], xP[:, 2 * C + c0 : 2 * C + c1])
        kblocks = [(3, 4), (4, 5), (5, 6), (6, 7)] + [
            (k, min(k + 2, KT)) for k in range(7, KT, 2)
        ]
        for bi, (ka, kb) in enumerate(kblocks):
            xeng, weng = (nc.sync, nc.scalar) if bi % 2 == 0 else (nc.scalar, nc.sync)
            weng.dma_start(wH[:, ka:kb], w0Hk[:, ka:kb])
            xeng.dma_start(
                hA[:, ka:kb, :],
                xP[:, ka * C : kb * C].rearrange("p (k c) -> p k c", k=kb - ka),
            )
        nc.sync.dma_start(bias_sb[:], bias[:])

        def relu_bias(out_ap, ps_ap, b_ap, on_dve):
            if on_dve:
                nc.vector.tensor_scalar(
                    out_ap, ps_ap, b_ap, 0.0,
                    mybir.AluOpType.add, mybir.AluOpType.max,
                )
            else:
                nc.scalar.activation(
                    out_ap, ps_ap,
                    mybir.ActivationFunctionType.Relu, bias=b_ap,
                )

        # ---- layer 0, phase A: first 8 (m,n) tiles, k-outer across all 8
        # PSUM banks; each x chunk is consumed by 8 matmuls as it lands ----
        pa_tiles = []          # (m, n) in n-inner order, capped at 8 banks
        for m in range(min(MT, MH)):   # phase A only has wH stripes m0..MH-1
            for n in range(NT):
                if len(pa_tiles) < 8:
                    pa_tiles.append((m, n))
        pa_ms = sorted({m for m, _ in pa_tiles})
        psA = {
            (m, n): pspool.tile([PT, TN], F32, tag="ps", name=f"psA_{m}_{n}")
            for m, n in pa_tiles
        }
        # n-outer within k so the first matmuls gate on the small k0/n0 piece
        pa_order = sorted(pa_tiles, key=lambda t: (t[1], t[0]))
        for k in range(KT):
            for m, n in pa_order:
                nc.tensor.matmul(
                    psA[(m, n)][:],
                    wH[:, k, m, :],
                    hA[:, k, bass.ts(n, TN)],
                    start=(k == 0),
                    stop=(k == KT - 1),
                    skip_group_check=True,
                )
        ei = 0
        for m, n in pa_tiles:
            relu_bias(hB[:, m, bass.ts(n, TN)], psA[(m, n)][:],
                      bias_sb[:, m : m + 1], ei % 2 == 1)
            ei += 1

        # ---- layer 0, phase B: remaining tiles k-inner (x resident) ----
        # finish partially-covered phase-A m's, then the rest
        done = set(pa_tiles)
        for m in pa_ms:
            for n in range(NT):
                if (m, n) in done:
                    continue
                ps = pspool.tile([PT, TN], F32, tag="ps", name=f"ps0_{m}_{n}")
                for k in range(KT):
                    nc.tensor.matmul(
                        ps[:], wH[:, k, m, :],
                        hA[:, k, bass.ts(n, TN)],
                        start=(k == 0), stop=(k == KT - 1),
                    )
                relu_bias(hB[:, m, bass.ts(n, TN)], ps[:],
                          bias_sb[:, m : m + 1], ei % 2 == 1)
                ei += 1
        for m in range(len(pa_ms), MT):
            wt = wpool.tile([PT, D], F16, tag="wt", name=f"wt0_{m}")
            nc.sync.dma_start(wt[:], w0[m])
            for n in range(NT):
                ps = pspool.tile([PT, TN], F32, tag="ps", name=f"ps0_{m}_{n}")
                for k in range(KT):
                    nc.tensor.matmul(
                        ps[:], wt[:, k * PT : (k + 1) * PT],
                        hA[:, k, bass.ts(n, TN)],
                        start=(k == 0), stop=(k == KT - 1),
                    )
                relu_bias(hB[:, m, bass.ts(n, TN)], ps[:],
                          bias_sb[:, m : m + 1], (n + m) % 2 == 1)

        # ---- layers 1 and 2 ----
        for w_dram, li, h_in, h_out in ((w1, 1, hB, hA), (w2, 2, hA, None)):
            for m in range(MT):
                wt = wpool.tile([PT, D], F16, tag="wt", name=f"wt{li}_{m}")
                nc.sync.dma_start(wt[:], w_dram[m])
                b_ap = bias_sb[:, li * MT + m : li * MT + m + 1]
                last_m = li == 2 and m == MT - 1
                fine_m = li == 2 and m >= MT - 3
                if h_out is None and not fine_m:
                    ot = opool.tile([PT, NT * TN], F16, tag="ot", name=f"ot{m}")
                for n in range(NT):
                    if last_m and n == NT - 1:
                        # kernel tail: final tile split unevenly; the last
                        # piece is small and rides a by-then-empty ring so
                        # the end-of-stream chain (epilogue + trigger + DMA)
                        # is as short as possible
                        cut = max(TN - 32, TN // 2)
                        for hi, (c0, c1) in enumerate(((0, cut), (cut, TN))):
                            psH = pspool.tile(
                                [PT, c1 - c0], F32, tag="ps", name=f"ps_last{hi}"
                            )
                            for k in range(KT):
                                nc.tensor.matmul(
                                    psH[:], wt[:, k * PT : (k + 1) * PT],
                                    h_in[:, k, n * TN + c0 : n * TN + c1],
                                    start=(k == 0), stop=(k == KT - 1),
                                )
                            otH = opool.tile([PT, c1 - c0], F16, tag="ot",
                                             name=f"ot_last{hi}")
                            relu_bias(otH[:], psH[:], b_ap, hi == 1)
                            dma_eng = nc.scalar if hi == 0 else nc.sync
                            dma_eng.dma_start(
                                yS[:, m, n * TN + c0 : n * TN + c1], otH[:]
                            )
                        continue
                    ps = pspool.tile([PT, TN], F32, tag="ps", name=f"ps{li}_{m}_{n}")
                    for k in range(KT):
                        nc.tensor.matmul(
                            ps[:], wt[:, k * PT : (k + 1) * PT],
                            h_in[:, k, bass.ts(n, TN)],
                            start=(k == 0), stop=(k == KT - 1),
                        )
                    on_dve = (n + m) % 2 == 1
                    if h_out is not None:
                        relu_bias(h_out[:, m, bass.ts(n, TN)], ps[:], b_ap, on_dve)
                    elif fine_m:
                        # per-tile out-DMA near the end so the final y data
                        # drains early instead of piling into the tail
                        otN = opool.tile([PT, TN], F16, tag="ot", name=f"otN{m}_{n}")
                        relu_bias(otN[:], ps[:], b_ap, on_dve)
                        dma_eng = nc.scalar if (last_m and n == 1) else nc.sync
                        dma_eng.dma_start(yS[:, m, bass.ts(n, TN)], otN[:])
                    else:
                        relu_bias(ot[:, bass.ts(n, TN)], ps[:], b_ap, on_dve)
                if h_out is None and not fine_m:
                    # per-m fused out-DMA: one [128, NT*TN*2B] contiguous run
                    nc.sync.dma_start(yS[:, m, :], ot[:])
    nc.compile()
    return nc


def _apportion_cores(counts):
    """Assign 8 cores to 4 leaves ~proportionally to token counts.
    Returns list of core counts per leaf (sums to N_CORES; 0 only for empty
    leaves). Greedy: repeatedly hand a core to the leaf with max load/core."""
    alive = [l for l in range(4) if counts[l] > 0]
    n = {l: 1 for l in alive}
    for _ in range(N_CORES - len(alive)):
        l = max(alive, key=lambda l: counts[l] / n[l])
        n[l] += 1
    return [n.get(l, 0) for l in range(4)]


def kernel(x, W0, b0, W1, b1, W2, b2, path_mask):
    global last_results
    x = np.asarray(x, dtype=np.float32)
    path_mask = np.asarray(path_mask)
    W0, b0, W1, b1, W2, b2 = (
        np.asarray(a, dtype=np.float32) for a in (W0, b0, W1, b1, W2, b2)
    )
    B = x.shape[0]

    bit0 = path_mask[:, 0].astype(np.int64)
    bit1 = path_mask[:, 1].astype(np.int64)
    leaf = 2 * bit0 + bit1
    order = np.argsort(leaf, kind="stable")
    counts = np.bincount(leaf, minlength=4)

    per_leaf = _apportion_cores(counts)
    # contiguous chunks of the leaf-sorted order per core
    groups = []      # list of (leaf, index-array) per core
    start = 0
    for l in range(4):
        cnt = int(counts[l])
        tok = order[start : start + cnt]
        start += cnt
        nl = per_leaf[l]
        if nl == 0:
            continue
        bounds = [round(i * cnt / nl) for i in range(nl + 1)]
        for i in range(nl):
            groups.append((l, tok[bounds[i] : bounds[i + 1]]))
    while len(groups) < N_CORES:  # only if some leaf was empty and slots remain
        groups.append((0, np.zeros(0, dtype=np.int64)))

    maxg = max(len(g[1]) for g in groups)
    TN, NT, C = _tiling(maxg)

    if C not in _compiled:
        _compiled[C] = _build(C, TN, NT)
    nc = _compiled[C]

    w_prepped = {}  # cache per (matrix id)
    def wp(tag, W):
        if tag not in w_prepped:
            w_prepped[tag] = _prep_weight(W)
        return w_prepped[tag]

    w0p = wp("w0", W0)
    # w0H[p, (k*MH + m)*PT + c] = w0p[m, p, k*PT + c]
    w0H = np.ascontiguousarray(
        w0p[:MH].reshape(MH, PT, KT, PT).transpose(1, 2, 0, 3).reshape(PT, KT * MH * PT)
    )
    xb = x.astype(NP_F16)
    in_maps = []
    for l, tok in groups:
        xTg = np.zeros((D, C), dtype=NP_F16)
        if len(tok):
            xTg[:, : len(tok)] = xb[tok].T
        xPg = np.ascontiguousarray(
            xTg.reshape(KT, PT, C).transpose(1, 0, 2).reshape(PT, KT * C)
        )
        in_maps.append(
            {
                "xP": xPg,
                "w0H": w0H,
                "w0": w0p,
                "w1": wp(("w1", l // 2), W1[l // 2]),
                "w2": wp(("w2", l), W2[l]),
                "bias": _prep_bias(b0, b1[l // 2], b2[l]),
            }
        )

    last_results = run_bass_kernel_spmd(nc, in_maps, core_ids=list(range(N_CORES)))

    y = np.empty((B, D), dtype=np.float32)
    for (l, tok), res in zip(groups, last_results.results):
        if len(tok):
            # [PT, MT, NT*TN] -> [D, C]
            yT = res["yS"].transpose(1, 0, 2).reshape(D, C)
            y[tok] = yT[:, : len(tok)].T.astype(np.float32)
    return y
